# revision 41
# baseline (speedup 1.0000x reference)
"""Trainium2 Bass kernel for BitConv2d:
GroupNorm(8) -> ReLU^2 -> PACT 8-bit quant -> 3x3 conv (ternary weight) -> bias.

Strategy (data-parallel over batch, 8 cores x 4 images), fp8 DoubleRow conv:
 - Host: ternarize the [256,256,3,3] weight; keep the {-1,0,+1} pattern as
   fp8e4 (exact), with alpha_oc/S folded into a per-out-channel rescale
   applied at PSUM evacuation.
 - Device per image: GroupNorm stats via bn_stats + two tiny PE matmuls
   (group-reduce and gamma-folded broadcast), then ACT relu(A*x+B) -> f16,
   ACT Square(sqrt(S)*v) in place, ACT Identity(u+1792) into a 58x58-padded
   f16 tile whose fp32->f16 convert rounds to the integer grid (ULP=1 on
   [1792,2047]; 1792 is even so RNE tie parity matches jnp.round; borders
   memset to 1792 represent n=0).
 - 8-bit-exact fp8 split: n = 16*hi + lo with hi,lo in [0,15].
   lo8 = mod(nf16, 16) -> fp8e4 (1792 % 16 == 0 so the offset drops out);
   hi16 = (nf16 - 1792) - lo8 -> fp8e4 (multiples of 16 <= 240, exact).
 - Conv: out[co, 8x58 flat span] accumulates 18 DoubleRow fp8 matmuls
   (9 taps x 2 cin chunks); the DoubleRow pair dim strides between the hi16
   and lo8 planes with the SAME ternary weight in both slots, so the PE
   computes w*(16*hi+lo) = w*n exactly at 0.5 cycles/row (2x fp16 rate).
   Flat 464-col spans keep the moving AP 3-level ([K,2,464]); the 2 garbage
   cols per row are skipped at evacuation (ACT Identity(psum*scale+bias)).
All quantities are small integers exactly representable in fp8e4/f32-PSUM,
so the conv is bit-exact integer arithmetic.
"""

import os
import sys

import numpy as np

for _p in ("/opt/trn_rl_repo", "/root/.axon_site/_ro/trn_rl_repo"):
    if _p not in sys.path and os.path.isdir(_p):
        sys.path.append(_p)

GN_GROUPS = 8
GN_EPS = 1e-5
K_BITS = 8
DELTA = 0.05
EPS = 1e-8

B_TOT, C, H, W = 32, 256, 56, 56
HW = H * W  # 3136
PW = W + 2  # 58
PHW = PW * PW  # 3364
PSTRIDE = 3392  # hi/lo plane stride (>=3364, multiple of 16)
N_CORES = 8
B_LOC = B_TOT // N_CORES  # 4
RB = 7          # row blocks per image
RBH = H // RB   # 8 rows per block
NN = RBH * W    # 448 valid conv outputs per row block
FLAT = RBH * PW  # 464 flat matmul columns per row block (8 rows x 58)
OFF = 1792.0    # f16 integer-rounding offset; multiple of 16 and even


def _host_prep(gamma, beta, a, weight_fp, bias):
    """Ternarize weights and pack all small device constants (numpy)."""
    import ml_dtypes

    w = weight_fp.astype(np.float32)
    wv = w.reshape(C, -1)
    wa = np.abs(wv)
    t = (DELTA * wa.mean(axis=1, keepdims=True)).astype(np.float32)
    m = (wa > t)
    alpha = ((wa * m).sum(axis=1, dtype=np.float64)
             / (m.sum(axis=1).astype(np.float64) + EPS)).astype(np.float32)
    tern = (np.sign(wv) * m).astype(np.float32)  # [-1, 0, 1]

    a_c = np.float32(max(float(a), 0.0) + EPS)
    S = np.float32((2.0 ** K_BITS - 1.0) / a_c)
    sqrt_s = float(np.float32(np.sqrt(np.float64(S))))

    # out = (alpha_co / S) * conv(n, tern) + bias_co; zero padding, no offset
    scale = (alpha / S).astype(np.float32)
    bias2 = bias.astype(np.float32).copy()

    # lhsT layout for DoubleRow: wt[p, k, q, c, s, m] with the SAME ternary
    # weight in both pair slots s (pair slots carry 16*hi and lo of the same
    # cin, so w0 == w1 == tern[oc, cin, k])
    tern4 = tern.reshape(2, 128, 2, 128, 9)          # [q, m, c, p, k]
    wt = np.zeros((128, 9, 2, 2, 2, 128), np.float32)
    wkqcm = tern4.transpose(3, 4, 0, 2, 1)           # [p, k, q, c, m]
    wt[:, :, :, :, 0, :] = wkqcm
    wt[:, :, :, :, 1, :] = wkqcm
    wt8 = wt.astype(ml_dtypes.float8_e4m3)

    # one packed [128, 268] f32 constant tensor -> a single DMA:
    #   cols 0:4     ind (group-average matrix, 1/32 one-hot)
    #   cols 4:132   indtg chunk0 (rows 0..3 used: gamma-weighted one-hot.T)
    #   cols 132:260 indtg chunk1 (rows 0..3 used)
    #   cols 260:262 sc chunk0 (scale, bias')   cols 262:264 sc chunk1
    #   cols 264:266 gb chunk0 (gamma, beta)    cols 266:268 gb chunk1
    # pack layout (f32 [128, 528]):
    #   cols 0:4     ind (group-average matrix, 1/32 one-hot)
    #   cols 4:132   rows 0..3: gamma-weighted one-hot.T (chunk0)
    #   cols 132:260 same for chunk1
    #   cols 260:264 sc: (scale, bias') per chunk
    #   cols 264:266 one-zero row on partition 0 (beta matmul rhs)
    #   cols 266:394 beta chunk0 on partition 0 (the GN-broadcast matmul
    #                accumulates beta so it directly produces
    #                (beta - gamma*mean*rsqrt, gamma*rsqrt))
    #   cols 394:522 beta chunk1 on partition 0
    #   cols 522:528 zero pad (cols 0:464 also feed PE warm-up dummies)
    g32 = gamma.astype(np.float32)
    pack = np.zeros((128, 528), np.float32)
    pack[np.arange(128), np.arange(128) // 32] = 1.0 / 32.0
    for cch in range(2):
        for p in range(128):
            pack[p // 32, 4 + 128 * cch + p] = g32[cch * 128 + p]
        pack[0, 266 + 128 * cch: 394 + 128 * cch] = (
            beta.astype(np.float32)[128 * cch: 128 * (cch + 1)])
        pack[:, 260 + 2 * cch] = scale.reshape(2, 128)[cch]
        pack[:, 261 + 2 * cch] = bias2.reshape(2, 128)[cch]
    pack[0, 264] = 1.0

    return dict(wt=wt8, pack=pack, sqrt_s=sqrt_s)


def _build_nc(sqrt_s, ablate=None):
    import concourse.bacc as bacc
    import concourse.mybir as mybir
    import concourse.tile as tile
    from contextlib import ExitStack

    f32 = mybir.dt.float32
    f16 = mybir.dt.float16
    f8 = mybir.dt.float8e4
    AF = mybir.ActivationFunctionType
    ALU = mybir.AluOpType
    DR = mybir.MatmulPerfMode.DoubleRow

    # Bacc (not raw Bass): its compile() runs generate_event_semaphores,
    # which legalizes multi-sem waits down to the 1-wait-per-instruction
    # TRN2 ISA constraint.
    nc = bacc.Bacc("TRN2", target_bir_lowering=False, debug=False,
                   num_devices=N_CORES)

    x_ap = nc.dram_tensor("x", [B_LOC, C, HW], f32, kind="ExternalInput").ap()
    wt_ap = nc.dram_tensor("wt", [128, 9, 2, 2, 2, 128], f8,
                           kind="ExternalInput").ap()
    pack_ap = nc.dram_tensor("pack", [128, 528], f32,
                             kind="ExternalInput").ap()
    out_ap = nc.dram_tensor("out", [B_LOC, C, HW], f32,
                            kind="ExternalOutput").ap()

    with tile.TileContext(nc) as tc, ExitStack() as ctx:
        consts = ctx.enter_context(tc.tile_pool(name="consts", bufs=1))
        x_pool = ctx.enter_context(tc.tile_pool(name="xp", bufs=2))
        u_pool = ctx.enter_context(tc.tile_pool(name="up", bufs=2))
        pad_pool = ctx.enter_context(tc.tile_pool(name="padp", bufs=1))
        st_pool = ctx.enter_context(tc.tile_pool(name="stp", bufs=2))
        tiny = ctx.enter_context(tc.tile_pool(name="tinyp", bufs=4))
        out_pool = ctx.enter_context(tc.tile_pool(name="outp", bufs=6))
        cps_pool = ctx.enter_context(tc.tile_pool(name="cps", bufs=5,
                                                  space="PSUM"))
        gps_pool = ctx.enter_context(tc.tile_pool(name="gps", bufs=1,
                                                  space="PSUM"))

        xs = [[None] * 2 for _ in range(B_LOC)]
        gms = [[None] * 2 for _ in range(B_LOC)]
        abs_ = [[None] * 2 for _ in range(B_LOC)]
        gns = {}

        def emit_load_chunk(b, cch, bounds=(0, HW)):
            # split loads (at bn_stats 448-block boundaries) so stats on
            # early blocks start before the tail lands (Tile subtile deps)
            xt = x_pool.tile([128, HW], f32, name=f"x_{b}_{cch}",
                             tag=f"x{cch}")
            for lo, hi in zip(bounds[:-1], bounds[1:]):
                nc.sync.dma_start(
                    out=xt[:, lo:hi],
                    in_=x_ap[b, cch * 128:(cch + 1) * 128, lo:hi])
            xs[b][cch] = xt

        def emit_load(b, bounds=(0, HW)):
            for cch in range(2):
                emit_load_chunk(b, cch, bounds)

        # pack dispatches first (tiny, feeds both the GN matmuls and the PE
        # warm-up dummies), then image 0's x pieces, then the conv weights
        # (first needed when chunk 0's first hi/lo bands are ready)
        pk_sb = consts.tile([128, 528], f32, name="pk_sb")
        nc.sync.dma_start(out=pk_sb, in_=pack_ap)
        emit_load_chunk(0, 0, bounds=(0, 3 * NN, 6 * NN, HW))
        emit_load_chunk(0, 1, bounds=(0, 3 * NN, 6 * NN, HW))
        w_sb = consts.tile([128, 9, 2, 2, 2, 128], f8, name="w_sb")
        nc.sync.dma_start(out=w_sb, in_=wt_ap)

        ind_sb = pk_sb[:, 0:4]
        indtg_sb = [pk_sb[0:4, 4:132], pk_sb[0:4, 132:260]]
        sc_sb = [pk_sb[:, 260:262], pk_sb[:, 262:264]]
        onez_sb = pk_sb[0:1, 264:266]
        beta_sb = [pk_sb[0:1, 266:394], pk_sb[0:1, 394:522]]
        eps_sb = consts.tile([4, 1], f32, name="eps_sb")
        nc.vector.memset(eps_sb, GN_EPS)

        # nf16: 58x58-padded integer image (value n + 1792, borders 1792)
        nf_t = [[pad_pool.tile([128, PW, PW], f16, name=f"nf_{cch}_{s}",
                               tag=f"nf{cch}{s}")
                 for s in range(2)] for cch in range(2)]
        # hi/lo fp8 planes: [:, 0, :] = 16*hi, [:, 1, :] = lo (pair dim for
        # the DoubleRow moving operand; plane stride PSTRIDE % 16 == 0)
        p8_t = [[pad_pool.tile([128, 2, PSTRIDE], f8, name=f"p8_{cch}_{s}",
                               tag=f"p8{cch}{s}")
                 for s in range(2)] for cch in range(2)]
        for cch in range(2):
            for s in range(2):
                # border-only memsets (interior is overwritten every image),
                # on GPSIMD which is otherwise busy only with lo-planes
                pt = nf_t[cch][s]
                nc.gpsimd.memset(pt[:, 0, :], OFF)
                nc.gpsimd.memset(pt[:, PW - 1, :], OFF)
                nc.gpsimd.memset(pt[:, 1:PW - 1, 0:1], OFF)
                nc.gpsimd.memset(pt[:, 1:PW - 1, PW - 1:PW], OFF)
                # hi/lo plane tails past PHW: the last row block's matmul
                # rhs slice reads 2 elements past the 58x58 image (garbage
                # columns, discarded at evac) — zero them once
                nc.gpsimd.memset(p8_t[cch][s][:, :, PHW:PHW + 8], 0.0)

        def emit_stats(b, chunks=(0, 1)):
            for cch in chunks:
                st6 = st_pool.tile([128, RB, 6], f32, name=f"st6_{b}_{cch}",
                                   tag="st6")
                for j in range(RB):
                    nc.vector.bn_stats(out=st6[:, j, :],
                                       in_=xs[b][cch][:, j * NN:(j + 1) * NN])
                # rhs = (mean, var, mean^2) built by slice-writes (keeps
                # every matmul-operand producer on DVE: Matmult's LDWEIGHTS
                # slot only fits 2 sync waits, walrus NCC_INLA001)
                m3 = tiny.tile([128, 3], f32, name=f"m3_{b}_{cch}", tag="m3")
                nc.vector.bn_aggr(out=m3[:, 0:2], in_=st6)
                nc.vector.tensor_mul(m3[:, 2:3], m3[:, 0:1], m3[:, 0:1])
                # gmv and bc share one PSUM tile (one bank), freeing a bank
                # for a 7-deep conv psum ring
                gn = gps_pool.tile([128, 8], f32, name=f"gn_{b}_{cch}",
                                   tag="gn")
                gmv = gn[0:4, 0:3]
                nc.tensor.matmul(gmv, lhsT=ind_sb, rhs=m3, start=True,
                                 stop=True)
                gm = tiny.tile([4, 3], f32, name=f"gm_{b}_{cch}", tag="gm")
                nc.vector.tensor_copy(out=gm, in_=gmv)
                gms[b][cch] = gm
                gns[(b, cch)] = gn

        def emit_post(b, chunks=(0, 1)):
            for cch in chunks:
                gm = gms[b][cch]
                gsq = tiny.tile([4, 1], f32, name=f"gsq_{b}_{cch}", tag="gsq")
                nc.vector.tensor_mul(gsq, gm[:, 0:1], gm[:, 0:1])
                gvar = tiny.tile([4, 1], f32, name=f"gvar_{b}_{cch}",
                                 tag="gvar")
                # var_g = (avg var + avg mean^2) - mean_g^2, one fused op
                nc.vector.scalar_tensor_tensor(
                    out=gvar, in0=gm[:, 1:2], scalar=gm[:, 2:3], in1=gsq,
                    op0=ALU.add, op1=ALU.subtract)
                # gvar <- sqrt(var + eps)
                nc.scalar.activation(out=gvar, in_=gvar, func=AF.Sqrt,
                                     bias=eps_sb, scale=1.0)
                vals = tiny.tile([4, 2], f32, name=f"vals_{b}_{cch}",
                                 tag="vals")
                nc.vector.reciprocal(out=vals[:, 1:2], in_=gvar)
                # vals0 = -mean_g * rsqrt_g (negated so the broadcast matmul
                # below can accumulate beta and yield the relu bias directly)
                nc.vector.scalar_tensor_tensor(
                    out=vals[:, 0:1], in0=gm[:, 0:1], scalar=-1.0,
                    in1=vals[:, 1:2], op0=ALU.mult, op1=ALU.mult)
                bc = gns[(b, cch)][:, 4:6]
                # ab = (beta - gamma*mean*rsqrt, gamma*rsqrt): the relu's
                # (bias, scale) with no further fixup ops
                nc.tensor.matmul(bc, lhsT=indtg_sb[cch], rhs=vals,
                                 start=True, stop=False)
                nc.tensor.matmul(bc, lhsT=beta_sb[cch], rhs=onez_sb,
                                 start=False, stop=True)
                ab = tiny.tile([128, 2], f32, name=f"ab_{b}_{cch}", tag="ab")
                nc.vector.tensor_copy(out=ab, in_=bc)
                abs_[b][cch] = ab

        # row-band wavefront: every stage is split into 4 bands so the
        # ACT -> Pool -> DVE -> Pool -> DVE chain pipelines at band
        # granularity; per-image chain latency stays well under the
        # per-image conv time, keeping the PE continuously fed (and at
        # full clock: the cost model's pstate ramp restarts on PE idle)
        act_tiles = {}
        # x-row ranges per piece; image 0 gets a small first band so its
        # first hi/lo rows (all conv rb0 needs) land ~2.5us sooner
        # (a smaller LAST band was tried to shorten the transition chain
        # tail, but the tile scheduler's reordering made it a net loss)
        PIECES = [(0, 14), (14, 28), (28, 42), (42, 56)]
        PIECES0 = [(0, 9), (9, 24), (24, 40), (40, 56)]

        def act_piece(b, cch, h, nh=4):
            ra, rb_ = (PIECES0 if b == 0 else PIECES)[h]
            if (b, cch) not in act_tiles:
                act_tiles[(b, cch)] = (
                    u_pool.tile([128, HW], f16, name=f"u_{b}_{cch}",
                                tag=f"u{cch}"),
                    u_pool.tile([128, PHW], f16, name=f"hf_{b}_{cch}",
                                tag=f"hf{cch}"))
            u, hf = act_tiles[(b, cch)]
            ab = abs_[b][cch]
            nf = nf_t[cch][b % 2]
            p8 = p8_t[cch][b % 2]
            nfflat = nf.rearrange("p h w -> p (h w)")
            sl = slice(ra * W, rb_ * W)
            uv = u[:, sl]
            # u = relu(A*x + B)  (f16; ~1e-3 rel on u, harmless: it only
            # moves u across round-to-int ties)
            nc.scalar.activation(out=uv, in_=xs[b][cch][:, sl],
                                 func=AF.Relu, bias=ab[:, 0:1],
                                 scale=ab[:, 1:2])
            # u = (sqrt(S)*u)^2 = S*relu(z)^2, in place (two-tensor DVE ops
            # cost 1x in the cost model, so ACT it is)
            nc.scalar.activation(out=uv, in_=uv, func=AF.Square,
                                 scale=sqrt_s)
            # nf16 interior = min(u, 255) + 1792 (min IS the PACT clip);
            # the f16 output convert rounds to the integer grid (RNE, ULP 1
            # on [1792, 2047], even offset preserves jnp.round tie parity)
            nc.gpsimd.tensor_scalar(
                out=nf[:, 1 + ra: 1 + rb_, 1:W + 1],
                in0=uv.rearrange("p (h w) -> p h w", h=rb_ - ra),
                scalar1=255.0, scalar2=OFF, op0=ALU.min, op1=ALU.add)
            # band = this piece's interior rows plus the memset border
            # row 0 (first band) / row 57 (last band)
            r0 = 0 if h == 0 else 1 + ra
            r1 = PW if h == nh - 1 else 1 + rb_
            bs = slice(r0 * PW, r1 * PW)
            # hf = RNE(nf/16 + 912.53125) = 1025 + hi exactly (the -7.5/16
            # centering makes RNE act as floor(n/16); values in
            # [1024.53, 1040.47] sit in the f16 ULP-1 zone, no ties)
            nc.vector.tensor_scalar(
                out=hf[:, bs], in0=nfflat[:, bs],
                scalar1=0.0625, scalar2=912.53125,
                op0=ALU.mult, op1=ALU.add)
            # hi plane: (hf - 1025) * 16 (multiples of 16, exact in fp8e4
            # up to 240); borders come out 0 with no memset
            nc.gpsimd.tensor_scalar(
                out=p8[:, 0, bs], in0=hf[:, bs],
                scalar1=-1025.0, scalar2=16.0,
                op0=ALU.add, op1=ALU.mult)
            # lo plane: (nf - 1792) - 16*hi = n mod 16 in [0, 15]
            nc.vector.scalar_tensor_tensor(
                out=p8[:, 1, bs], in0=nfflat[:, bs], scalar=-OFF,
                in1=p8[:, 0, bs], op0=ALU.add, op1=ALU.subtract)

        def emit_act(b, split=4, chunks=(0, 1)):
            for cch in chunks:
                for h in range(split):
                    act_piece(b, cch, h, split)

        def emit_conv(b, q, deep=False, slots=None):
            # one full-image output tile per (b, q): evacs write slices and a
            # single big DMA stores it.  deep=True (image 0) runs chunk 0's
            # taps for the first row blocks first (open PSUM groups), so the
            # PE can start ~5us before chunk 1's hi/lo planes are ready.
            # `slots` is a queue of emission thunks (stats/post/act pieces/
            # loads for later images) consumed one per evacuation, so each
            # engine's FIFO receives cross-image work in dependency-ready
            # order instead of in big blocking batches.
            osb = out_pool.tile([128, HW], f32, name=f"o_{b}_{q}", tag="osb",
                                bufs=2)
            ps = [None] * RB

            def taps(rb, cch, i0):
                i = i0
                for k in range(9):
                    dy, dx = divmod(k, 3)
                    off = (rb * RBH + dy) * PW + dx
                    nc.tensor.matmul(
                        ps[rb],
                        lhsT=w_sb[:, k, q, cch],
                        rhs=p8_t[cch][b % 2][:, :, off:off + FLAT],
                        start=(i == 0), stop=(i == 17),
                        perf_mode=DR)
                    i += 1

            def evac(rb):
                # skip the 2 garbage columns per flat row at evacuation.
                # A few evacs run on DVE tensor_scalar (psum*scale+bias with
                # per-partition AP scalars) to keep ACT under the PE time.
                src = ps[rb].rearrange("p (h w) -> p h w", h=RBH)[:, :, 0:W]
                dst = osb[:, rb * NN:(rb + 1) * NN]
                if q == 1 and rb in (1, 3):
                    # two q1 evacs on DVE keep ACT under the PE time, but
                    # never the late row blocks: a DVE evac there lands in
                    # the middle of the next image's hf/lo chain tail and
                    # stalls the PE at the image transition
                    nc.vector.tensor_scalar(
                        out=dst, in0=src,
                        scalar1=sc_sb[q][:, 0:1], scalar2=sc_sb[q][:, 1:2],
                        op0=ALU.mult, op1=ALU.add)
                else:
                    nc.scalar.activation(
                        out=dst, in_=src, func=AF.Identity,
                        bias=sc_sb[q][:, 1:2],
                        scale=sc_sb[q][:, 0:1])

            DEEP = 6 if deep else 1
            for rb in range(DEEP):
                ps[rb] = cps_pool.tile([128, FLAT], f32,
                                       name=f"ps_{b}_{q}_{rb}",
                                       tag="cps", bufs=7)
                taps(rb, 0, 0)
            for rb in range(RB):
                if rb >= DEEP:
                    ps[rb] = cps_pool.tile([128, FLAT], f32,
                                           name=f"ps_{b}_{q}_{rb}",
                                           tag="cps", bufs=7)
                    taps(rb, 0, 0)
                taps(rb, 1, 9)
                evac(rb)
                if slots:
                    slots.pop(0)()
            # piecewise stores: each piece only depends on the evacs that
            # wrote it (subtile deps).  The last image streams out
            # per-rowblock so the kernel tail isn't gated on one big DMA.
            cuts = (list(range(0, HW + 1, NN)) if b == B_LOC - 1
                    else [0, 4 * NN, HW])
            for lo, hi in zip(cuts[:-1], cuts[1:]):
                nc.sync.dma_start(out=out_ap[b, q * 128:(q + 1) * 128, lo:hi],
                                  in_=osb[:, lo:hi])

        if ablate == "conv":
            for b in range(B_LOC):
                emit_conv(b, 0)
                emit_conv(b, 1)
        elif ablate == "gn":
            emit_stats(0); emit_post(0); emit_act(0)
            for b in range(1, B_LOC):
                emit_load(b); emit_stats(b); emit_post(b); emit_act(b)
        else:
            # image 0's chain runs chunk-at-a-time for minimum fill latency.
            # All later-image work (stats/post, act pieces, x loads) is
            # queued as slot thunks consumed at conv evacuation boundaries:
            # image b+1's act wavefront spreads across conv(b), image b+2's
            # stats/post and loads ride the tail slots, and no engine FIFO
            # ever holds a not-yet-ready instruction in front of ready conv
            # work (which would idle the PE and drop it out of full clock).
            for cch in range(2):
                emit_stats(0, chunks=(cch,))
                emit_post(0, chunks=(cch,))
                emit_act(0, chunks=(cch,))
            emit_load(1, bounds=(0, 4 * NN, HW))

            def sp(bb, cch):
                def f():
                    emit_stats(bb, chunks=(cch,))
                    emit_post(bb, chunks=(cch,))
                return f

            def ap(bb, cch, h):
                return lambda: act_piece(bb, cch, h)

            def ld(bb):
                return lambda: emit_load(bb, bounds=(0, 4 * NN, HW))

            for b in range(B_LOC):
                slots = []
                if b + 1 < B_LOC:
                    if b == 0:
                        slots.append(sp(1, 0))
                    slots += [ap(b + 1, 0, h) for h in range(4)]
                    if b == 0:
                        slots.append(sp(1, 1))
                    slots += [ap(b + 1, 1, h) for h in range(4)]
                if b + 2 < B_LOC:
                    slots.append(ld(b + 2))
                    slots.append(sp(b + 2, 0))
                    slots.append(sp(b + 2, 1))
                emit_conv(b, 0, deep=(b == 0), slots=slots)
                emit_conv(b, 1, slots=slots)
                while slots:
                    slots.pop(0)()

    nc.compile()
    return nc


def kernel(x, gamma, beta, a, weight_fp, bias):
    consts = _host_prep(np.asarray(gamma), np.asarray(beta), np.asarray(a),
                        np.asarray(weight_fp), np.asarray(bias))
    nc = _build_nc(consts.pop("sqrt_s"))

    from concourse.bass_utils import run_bass_kernel_spmd

    x = np.ascontiguousarray(np.asarray(x, dtype=np.float32)
                             .reshape(B_TOT, C, HW))
    in_maps = []
    for core in range(N_CORES):
        in_maps.append({
            "x": x[core * B_LOC:(core + 1) * B_LOC],
            "wt": consts["wt"],
            "pack": consts["pack"],
        })
    res = run_bass_kernel_spmd(nc, in_maps, list(range(N_CORES)))
    out = np.concatenate([res.results[i]["out"] for i in range(N_CORES)],
                         axis=0)
    return out.reshape(B_TOT, C, H, W)


if __name__ == "__main__":
    rng = np.random.default_rng(0)
    x = rng.standard_normal((B_TOT, C, H, W), dtype=np.float32)
    out = kernel(x, np.ones(C, np.float32), np.zeros(C, np.float32),
                 np.float32(6.0),
                 rng.standard_normal((C, C, 3, 3), dtype=np.float32) * 0.03,
                 np.zeros(C, np.float32))
    print(out.shape, out.dtype)
